# revision 49
# baseline (speedup 1.0000x reference)
"""Trainium2 Bass kernel for nn_Distiller (attention-transfer distillation loss).

Computes on 8 NeuronCores (data-parallel over batch, 2 batches/core):
  SA part: per batch, weighted spatial grams A = V^T V with V = F * sqrt(Fc)
           (Fc = sum |F| over space), for teacher/student features; then
           rho_m = <A_S[m], A_T[m]> / (|A_S[m]| |A_T[m]|) per row.
           Uses the identity sum_n (Ahat_S - Ahat_T)^2 = 2 - 2*rho per row.
  IC part: per batch, channel grams G = L L^T of the [21, 16384] logit maps,
           same rho identity on the 21 rows of G.
Device emits per-row rho partial sums; host assembles the two scalar losses.
s_out passes through on host.
"""

import sys

if "/opt/trn_rl_repo" not in sys.path:
    sys.path.insert(0, "/opt/trn_rl_repo")

import numpy as np
import ml_dtypes

import concourse.bass as bass
import concourse.mybir as mybir
import concourse.tile as tile
from concourse.bass_utils import run_bass_kernel_spmd

# Problem shapes (hardcoded per spec)
B = 16
C = 512
M = 1024  # 32*32 spatial
CC = 21
M2 = 16384  # 128*128 spatial
N_CORES = 8
BPC = B // N_CORES  # batches per core = 2

FP = mybir.dt.float32
FPR = mybir.dt.float32r
BF = mybir.dt.bfloat16
AX = mybir.AxisListType.X
ALU = mybir.AluOpType
ACTF = mybir.ActivationFunctionType



def _split_sync_waits(nc, cap=1):
    """walrus in this container accepts at most `cap` sync waits per
    instruction; hoist excess waits onto same-engine NOPs just before."""
    n = 0
    for f in nc.m.functions:
        for bb in f.blocks:
            newlist = []
            for ins in bb.instructions:
                si = ins.sync_info
                if si is not None and si.on_wait and len(si.on_wait) > cap:
                    waits = list(si.on_wait)
                    hoist, keep = waits[:-cap], waits[-cap:]
                    for w in hoist:
                        n += 1
                        newlist.append(
                            mybir.InstNoOp(
                                name=f"waitsplit-{n}",
                                engine=ins.engine,
                                ins=[],
                                outs=[],
                                sync_info=mybir.SyncInfo(on_wait=[w], on_update=[]),
                            )
                        )
                    si.on_wait = keep
                newlist.append(ins)
            bb.instructions = newlist


def _build():
    nc = bass.Bass(trn_type="TRN2")
    # bf16 feature maps: halves the SA DMA bytes; V keeps f32r rounding
    tf = nc.dram_tensor("TF", [BPC, C, M], BF, kind="ExternalInput")
    sf = nc.dram_tensor("SF", [BPC, C, M], BF, kind="ExternalInput")
    # IC inputs: bf16 hi/lo split, both batches packed into 96 rows (dense:
    # hi_b0 0:21, lo_b0 21:42, hi_b1 42:63, lo_b1 63:84, zeros to 96; block
    # extraction happens on the host so no alignment is needed).
    to = nc.dram_tensor("TOHL", [96, M2], BF, kind="ExternalInput")
    so = nc.dram_tensor("SOHL", [96, M2], BF, kind="ExternalInput")
    ones_d = nc.dram_tensor("ONES", [128, 1], FP, kind="ExternalInput")
    ost = nc.dram_tensor("OST", [3, 128, 16], FP, kind="ExternalOutput")
    och = nc.dram_tensor("OCH", [BPC, 3, M], FP, kind="ExternalOutput")
    og = nc.dram_tensor("OG", [2, 96, 96], FP, kind="ExternalOutput")

    with tile.TileContext(nc) as tc:
        with (
            tc.tile_pool(name="icl", bufs=1) as picl,
            tc.tile_pool(name="vt", bufs=1) as pv,
            tc.tile_pool(name="fstg", bufs=8) as pf,
            tc.tile_pool(name="scr", bufs=3) as pscr,
            tc.tile_pool(name="stat", bufs=1) as pstat,
        ):
            ones_f = pstat.tile([128, 2], FP, name="ones_f")
            nc.sync.dma_start(ones_f[:, 0:1], ones_d[:])
            nc.sync.dma_start(ones_f[:, 1:2], ones_d[:])
            ones_r = pstat.tile([128, 2], BF, name="ones_r")
            nc.vector.tensor_copy(ones_r[:], ones_f[:])

            vmats = {}
            fcs = {}

            def prep_k(bi, tname, k, src):
                # F staged in f32, V = F * sqrt(Fc) written as float32r (the
                # rounding producer the FP32r matmul verifier requires).
                key = (tname, bi)
                if key not in fcs:
                    fc = pstat.tile([128, 4], FP, name=f"fc_{tname}_{bi}")
                    sfc = pstat.tile([128, 4], FP, name=f"sfc_{tname}_{bi}")
                    fcs[key] = (fc, sfc)
                fc, sfc = fcs[key]
                f = pf.tile([128, M], BF, name=f"fstg_{k}", tag="fstg")
                nc.sync.dma_start(f[:], src[bi, 128 * k : 128 * (k + 1), :])
                nc.vector.tensor_reduce(
                    out=fc[:, k : k + 1],
                    in_=f[:],
                    axis=AX,
                    op=ALU.add,
                    apply_absolute_value=True,
                )
                nc.scalar.activation(
                    sfc[:, k : k + 1], fc[:, k : k + 1], ACTF.Sqrt
                )
                v = pv.tile([128, M], FPR, name=f"v_{tname}_{bi}_{k}")
                nc.vector.tensor_scalar_mul(v[:], f[:], sfc[:, k : k + 1])
                vmats[(tname, bi, k)] = v

            def prep_batch(bi):
                for tname, src in (("t", tf), ("s", sf)):
                    for k in range(C // 128):
                        prep_k(bi, tname, k, src)

            ns2b = pstat.tile([128, 16], FP, name="ns2b")
            nt2b = pstat.tile([128, 16], FP, name="nt2b")
            stb = pstat.tile([128, 16], FP, name="stb")

            def sa_batch(bi, pps, tile_hook=None):
                # A = V^T V is symmetric: compute only columns n >= the m-tile
                # diagonal. Row sums of the skipped lower triangle equal
                # column sums of the upper part, accumulated via ones^T @ f(A)
                # matmul chains into `cs` (partition 0=prod, 32=sqS, 64=sqT);
                # host merges. cs cols 0:128 are never written (garbage) and
                # are masked on the host.
                cs = pps.tile([66, M], FP, name="cs", tag="cs", bufs=1)
                pending = []
                for mt in range(8):
                    # emit chains deferred by THREE tiles so PE never waits
                    # on recent stats (scr pool holds 4 generations)
                    while len(pending) > 2:
                        pending.pop(0)()
                    idx = bi * 8 + mt
                    W = M - 128 * mt
                    psa = pps.tile([128, M], FP, name="psa", tag="psa", bufs=1)
                    psb = pps.tile([128, M], FP, name="psb", tag="psb", bufs=2)
                    # pieces must not cross the 512-elem PSUM bank boundary
                    pieces = [(0, 512), (512, W - 512)] if W > 512 else [(0, W)]
                    for dst, tname in ((psa, "t"), (psb, "s")):
                        for off, nn in pieces:
                            for k in range(4):
                                v = vmats[(tname, bi, k)]
                                nc.tensor.matmul(
                                    dst[:, off : off + nn],
                                    lhsT=v[:, 128 * mt : 128 * (mt + 1)],
                                    rhs=v[:, 128 * mt + off : 128 * mt + off + nn],
                                    start=(k == 0),
                                    stop=(k == 3),
                                )
                    at_sb = pscr.tile([128, M], BF, name="at_sb", tag="atsb")
                    scr1 = pscr.tile([128, M], BF, name="scr1", tag="scr", bufs=12)
                    scr2 = pscr.tile([128, M], BF, name="scr2", tag="scr", bufs=12)
                    scr3 = pscr.tile([128, M], BF, name="scr3", tag="scr", bufs=12)
                    # psa's ONLY consumer is this fast cast (slot frees in
                    # ~0.5us); nt2/sq_t come from the bf16 SBUF copy instead.
                    nc.vector.tensor_copy(at_sb[:, 0:W], psa[:, 0:W])
                    nc.scalar.activation(
                        scr1[:, 0:W], psb[:, 0:W], ACTF.Square,
                        accum_out=ns2b[:, idx : idx + 1],
                    )
                    # psb-freeing product first; the PSUM-independent at^2
                    # stat last so slots recycle as early as possible
                    nc.vector.scalar_tensor_tensor(
                        out=scr3[:, 0:W],
                        in0=psb[:, 0:W],
                        scalar=1.0,
                        in1=at_sb[:, 0:W],
                        op0=ALU.mult,
                        op1=ALU.mult,
                        accum_out=stb[:, idx : idx + 1],
                    )
                    nc.vector.scalar_tensor_tensor(
                        out=scr2[:, 0:W],
                        in0=at_sb[:, 0:W],
                        scalar=1.0,
                        in1=at_sb[:, 0:W],
                        op0=ALU.mult,
                        op1=ALU.mult,
                        accum_out=nt2b[:, idx : idx + 1],
                    )
                    if tile_hook is not None:
                        tile_hook(mt)
                    if W > 128:
                        g0 = 128 * (mt + 1)
                        segs = []  # aligned to PSUM bank boundaries
                        for bb0 in (0, 512):
                            lo = max(g0, bb0)
                            if lo < bb0 + 512:
                                segs.append((lo, bb0 + 512))

                        def _mk(mt=mt, segs=segs, scrs=(scr3, scr1, scr2)):
                            def emit():
                                for srow, scr in zip((0, 32, 64), scrs):
                                    for a2, b2 in segs:
                                        la = a2 - 128 * mt
                                        nc.tensor.matmul(
                                            cs[srow : srow + 2, a2:b2],
                                            lhsT=ones_r[:, 0:2],
                                            rhs=scr[:, la : la + b2 - a2],
                                            start=(mt == 0),
                                            stop=(mt == 6),
                                            skip_group_check=True,
                                        )
                            return emit

                        pending.append(_mk())
                for p in pending:
                    p()
                csb = pstat.tile([66, M], FP, name=f"csb_{bi}")
                nc.scalar.copy(csb[:], cs[:])
                for s in range(3):
                    nc.sync.dma_start(och[bi, s], csb[32 * s : 32 * s + 1, :])

            # ---- phase order: prep b0, SA b0 | IC | prep b1 (DMA), SA b1.
            # DMA program order on SP: F-b0, IC transposes, F-b1 -> PE is
            # never data-starved.
            prep_batch(0)

            # batch-1 prep is interleaved into SA-b0's per-tile stats: its F
            # loads dispatch right after b0's (slot-gated), and its DVE/ACT
            # work slots between SA-b0 stat ops without head-of-line blocking.
            _prep1 = [(tn, k) for tn in ("t", "s") for k in range(C // 128)]

            def _hook(mt):
                tn, k = _prep1[mt]
                prep_k(1, tn, k, tf if tn == "t" else sf)

            with tc.tile_pool(name="saps0", bufs=2, space="PSUM") as pps0:
                sa_batch(0, pps0, tile_hook=_hook)

            # IC loads: hardware DMA-transpose, [128, 4096] chunks land as
            # [128p, 32blk, 128ch] with m = blk*128 + p.
            lmats = {}
            for tname, hl in (("t", to), ("s", so)):
                tt = picl.tile([128, 128, 128], BF, name=f"tt_{tname}")
                for c in range(4):
                    nc.sync.dma_start_transpose(
                        tt[:, 32 * c : 32 * (c + 1), :],
                        hl[:, 4096 * c : 4096 * (c + 1)],
                    )
                lmats[tname] = tt

            # ---------------- IC phase (own PSUM pool) ----------------
            # One matmul per (tensor, m-block): the [128, 118] product holds
            # hi/lo cross-grams of both batches. The [21,21] block extraction
            # G = hh + hl + hl^T + ll and the loss math happen on the host
            # (f64) from the shipped OG blocks.
            with tc.tile_pool(name="icg", bufs=1, space="PSUM") as pg:
                for ti, tname in enumerate(("t", "s")):
                    tt = lmats[tname]
                    psg = pg.tile([96, 96], FP, name=f"psg_{tname}")
                    for j in range(128):
                        nc.tensor.matmul(
                            psg[:, :],
                            lhsT=tt[:, j, 0:96],
                            rhs=tt[:, j, 0:96],
                            start=(j == 0),
                            stop=(j == 127),
                        )
                    gsb = pstat.tile([96, 96], FP, name=f"gsb_{tname}")
                    nc.scalar.copy(gsb[:], psg[:])
                    nc.sync.dma_start(og[ti], gsb[:])

            # ---------------- SA batch 1 ----------------
            with tc.tile_pool(name="saps1", bufs=2, space="PSUM") as pps1:
                sa_batch(1, pps1)

            # ship raw SA stats; host does rho math in f64
            nc.sync.dma_start(ost[0], stb[:])
            nc.sync.dma_start(ost[1], ns2b[:])
            nc.sync.dma_start(ost[2], nt2b[:])

    _split_sync_waits(nc)
    return nc


_NC = None


def _get_nc():
    global _NC
    if _NC is None:
        _NC = _build()
    return _NC


def _hl_pack(X):
    """[B, CC, M2] f32 -> per-core [128, M2] bf16 with rows
    0:21 b0-hi, 32:53 b0-lo, 64:85 b1-hi, 96:117 b1-lo."""
    bf = ml_dtypes.bfloat16
    hi = X.astype(bf)
    lo = (X - hi.astype(np.float32)).astype(bf)
    out = np.zeros((N_CORES, 96, M2), dtype=bf)
    out[:, 0:21] = hi[0::2]
    out[:, 21:42] = lo[0::2]
    out[:, 42:63] = hi[1::2]
    out[:, 63:84] = lo[1::2]
    return out


def _make_in_maps(TF, SF, t_out, s_out):
    bf = ml_dtypes.bfloat16
    TFr = np.ascontiguousarray(np.asarray(TF, dtype=np.float32).reshape(B, C, M).astype(bf))
    SFr = np.ascontiguousarray(np.asarray(SF, dtype=np.float32).reshape(B, C, M).astype(bf))
    TOhl = _hl_pack(np.asarray(t_out, dtype=np.float32).reshape(B, CC, M2))
    SOhl = _hl_pack(np.asarray(s_out, dtype=np.float32).reshape(B, CC, M2))
    in_maps = []
    for i in range(N_CORES):
        sl = slice(BPC * i, BPC * (i + 1))
        in_maps.append(
            {
                "TF": np.ascontiguousarray(TFr[sl]),
                "SF": np.ascontiguousarray(SFr[sl]),
                "TOHL": np.ascontiguousarray(TOhl[i]),
                "SOHL": np.ascontiguousarray(SOhl[i]),
                "ONES": np.ones((128, 1), dtype=np.float32),
            }
        )
    return in_maps


def _assemble(results, s_out):
    EPS = 1e-12
    sa_rho = 0.0
    ic_num = 0.0
    for r in results:
        st = r["OST"][0].astype(np.float64)
        ns2 = r["OST"][1].astype(np.float64)
        nt2 = r["OST"][2].astype(np.float64)
        for bi in range(BPC):
            sl = slice(8 * bi, 8 * (bi + 1))
            ch = r["OCH"][bi].astype(np.float64)
            ch[:, 0:128] = 0.0  # chain cols 0:128 are uninitialized PSUM
            st[:, sl] += ch[0].reshape(8, 128).T
            ns2[:, sl] += ch[1].reshape(8, 128).T
            nt2[:, sl] += ch[2].reshape(8, 128).T
        sa_rho += float((st / np.maximum(np.sqrt(ns2 * nt2), EPS)).sum())
        gpair = []
        for ti in range(2):  # t, s
            gsb = r["OG"][ti].astype(np.float64)
            gb = []
            for bi, (hh, hl, ll) in enumerate(
                (((0, 0), (0, 21), (21, 21)), ((42, 42), (42, 63), (63, 63)))
            ):
                Ghh = gsb[hh[0] : hh[0] + CC, hh[1] : hh[1] + CC]
                Ghl = gsb[hl[0] : hl[0] + CC, hl[1] : hl[1] + CC]
                Gll = gsb[ll[0] : ll[0] + CC, ll[1] : ll[1] + CC]
                gb.append(Ghh + Ghl + Ghl.T + Gll)
            gpair.append(gb)
        for bi in range(BPC):
            Gt, Gs = gpair[0][bi], gpair[1][bi]
            Gtn = Gt / np.maximum(
                np.linalg.norm(Gt, axis=1, keepdims=True), EPS
            )
            Gsn = Gs / np.maximum(
                np.linalg.norm(Gs, axis=1, keepdims=True), EPS
            )
            d = Gsn - Gtn
            ic_num += float((d * d).sum())
    sa_loss = (2.0 * B * M - 2.0 * sa_rho) / (B * M * M)
    ic_loss = ic_num / (CC * B)
    return (s_out, np.float32(ic_loss), np.float32(sa_loss))


def kernel(TF, SF, t_out, s_out, _trace=False):
    nc = _get_nc()
    in_maps = _make_in_maps(TF, SF, t_out, s_out)
    res = run_bass_kernel_spmd(nc, in_maps, core_ids=list(range(N_CORES)), trace=_trace)
    out = _assemble(res.results, s_out)
    if _trace:
        return out, res
    return out


# revision 50
# speedup vs baseline: 1.1225x; 1.1225x over previous
"""Trainium2 Bass kernel for nn_Distiller (attention-transfer distillation loss).

Computes on 8 NeuronCores (data-parallel over batch, 2 batches/core):
  SA part: per batch, weighted spatial grams A = V^T V with V = F * sqrt(Fc)
           (Fc = sum |F| over space), for teacher/student features; then
           rho_m = <A_S[m], A_T[m]> / (|A_S[m]| |A_T[m]|) per row.
           Uses the identity sum_n (Ahat_S - Ahat_T)^2 = 2 - 2*rho per row.
  IC part: per batch, channel grams G = L L^T of the [21, 16384] logit maps,
           same rho identity on the 21 rows of G.
Device emits per-row rho partial sums; host assembles the two scalar losses.
s_out passes through on host.
"""

import sys

if "/opt/trn_rl_repo" not in sys.path:
    sys.path.insert(0, "/opt/trn_rl_repo")

import numpy as np
import ml_dtypes

import concourse.bass as bass
import concourse.mybir as mybir
import concourse.tile as tile
from concourse.bass_utils import run_bass_kernel_spmd

# Problem shapes (hardcoded per spec)
B = 16
C = 512
M = 1024  # 32*32 spatial
CC = 21
M2 = 16384  # 128*128 spatial
N_CORES = 8
BPC = B // N_CORES  # batches per core = 2

FP = mybir.dt.float32
FPR = mybir.dt.float32r
BF = mybir.dt.bfloat16
AX = mybir.AxisListType.X
ALU = mybir.AluOpType
ACTF = mybir.ActivationFunctionType



def _split_sync_waits(nc, cap=1):
    """walrus in this container accepts at most `cap` sync waits per
    instruction; hoist excess waits onto same-engine NOPs just before."""
    n = 0
    for f in nc.m.functions:
        for bb in f.blocks:
            newlist = []
            for ins in bb.instructions:
                si = ins.sync_info
                if si is not None and si.on_wait and len(si.on_wait) > cap:
                    waits = list(si.on_wait)
                    hoist, keep = waits[:-cap], waits[-cap:]
                    for w in hoist:
                        n += 1
                        newlist.append(
                            mybir.InstNoOp(
                                name=f"waitsplit-{n}",
                                engine=ins.engine,
                                ins=[],
                                outs=[],
                                sync_info=mybir.SyncInfo(on_wait=[w], on_update=[]),
                            )
                        )
                    si.on_wait = keep
                newlist.append(ins)
            bb.instructions = newlist


def _build():
    nc = bass.Bass(trn_type="TRN2")
    # bf16 feature maps: halves the SA DMA bytes; V keeps f32r rounding
    tf = nc.dram_tensor("TF", [BPC, C, M], BF, kind="ExternalInput")
    sf = nc.dram_tensor("SF", [BPC, C, M], BF, kind="ExternalInput")
    # IC inputs: bf16 (hi-only; the cancellation-free D-form + f64 host math
    # keeps ic error ~2e-5). Rows 0:21 b0, 21:42 b1, zeros to 48.
    to = nc.dram_tensor("TOHL", [48, M2], BF, kind="ExternalInput")
    so = nc.dram_tensor("SOHL", [48, M2], BF, kind="ExternalInput")
    ones_d = nc.dram_tensor("ONES", [128, 1], FP, kind="ExternalInput")
    ost = nc.dram_tensor("OST", [3, 128, 16], FP, kind="ExternalOutput")
    och = nc.dram_tensor("OCH", [BPC, 3, M], FP, kind="ExternalOutput")
    og = nc.dram_tensor("OG", [2, 48, 48], FP, kind="ExternalOutput")

    with tile.TileContext(nc) as tc:
        with (
            tc.tile_pool(name="icl", bufs=1) as picl,
            tc.tile_pool(name="vt", bufs=1) as pv,
            tc.tile_pool(name="fstg", bufs=8) as pf,
            tc.tile_pool(name="scr", bufs=3) as pscr,
            tc.tile_pool(name="stat", bufs=1) as pstat,
        ):
            ones_f = pstat.tile([128, 2], FP, name="ones_f")
            nc.sync.dma_start(ones_f[:, 0:1], ones_d[:])
            nc.sync.dma_start(ones_f[:, 1:2], ones_d[:])
            ones_r = pstat.tile([128, 2], BF, name="ones_r")
            nc.vector.tensor_copy(ones_r[:], ones_f[:])

            vmats = {}
            fcs = {}

            def prep_k(bi, tname, k, src):
                # F staged in f32, V = F * sqrt(Fc) written as float32r (the
                # rounding producer the FP32r matmul verifier requires).
                key = (tname, bi)
                if key not in fcs:
                    fc = pstat.tile([128, 4], FP, name=f"fc_{tname}_{bi}")
                    sfc = pstat.tile([128, 4], FP, name=f"sfc_{tname}_{bi}")
                    fcs[key] = (fc, sfc)
                fc, sfc = fcs[key]
                f = pf.tile([128, M], BF, name=f"fstg_{k}", tag="fstg")
                nc.sync.dma_start(f[:], src[bi, 128 * k : 128 * (k + 1), :])
                nc.vector.tensor_reduce(
                    out=fc[:, k : k + 1],
                    in_=f[:],
                    axis=AX,
                    op=ALU.add,
                    apply_absolute_value=True,
                )
                nc.scalar.activation(
                    sfc[:, k : k + 1], fc[:, k : k + 1], ACTF.Sqrt
                )
                v = pv.tile([128, M], FPR, name=f"v_{tname}_{bi}_{k}")
                nc.vector.tensor_scalar_mul(v[:], f[:], sfc[:, k : k + 1])
                vmats[(tname, bi, k)] = v

            def prep_batch(bi):
                for tname, src in (("t", tf), ("s", sf)):
                    for k in range(C // 128):
                        prep_k(bi, tname, k, src)

            ns2b = pstat.tile([128, 16], FP, name="ns2b")
            nt2b = pstat.tile([128, 16], FP, name="nt2b")
            stb = pstat.tile([128, 16], FP, name="stb")

            def sa_batch(bi, pps, tile_hook=None):
                # A = V^T V is symmetric: compute only columns n >= the m-tile
                # diagonal. Row sums of the skipped lower triangle equal
                # column sums of the upper part, accumulated via ones^T @ f(A)
                # matmul chains into `cs` (partition 0=prod, 32=sqS, 64=sqT);
                # host merges. cs cols 0:128 are never written (garbage) and
                # are masked on the host.
                cs = pps.tile([66, M], FP, name="cs", tag="cs", bufs=1)
                pending = []
                for mt in range(8):
                    # emit chains deferred by THREE tiles so PE never waits
                    # on recent stats (scr pool holds 4 generations)
                    while len(pending) > 2:
                        pending.pop(0)()
                    idx = bi * 8 + mt
                    W = M - 128 * mt
                    psa = pps.tile([128, M], FP, name="psa", tag="psa", bufs=1)
                    psb = pps.tile([128, M], FP, name="psb", tag="psb", bufs=2)
                    # pieces must not cross the 512-elem PSUM bank boundary
                    pieces = [(0, 512), (512, W - 512)] if W > 512 else [(0, W)]
                    for dst, tname in ((psa, "t"), (psb, "s")):
                        for off, nn in pieces:
                            for k in range(4):
                                v = vmats[(tname, bi, k)]
                                nc.tensor.matmul(
                                    dst[:, off : off + nn],
                                    lhsT=v[:, 128 * mt : 128 * (mt + 1)],
                                    rhs=v[:, 128 * mt + off : 128 * mt + off + nn],
                                    start=(k == 0),
                                    stop=(k == 3),
                                )
                    at_sb = pscr.tile([128, M], BF, name="at_sb", tag="atsb")
                    scr1 = pscr.tile([128, M], BF, name="scr1", tag="scr", bufs=12)
                    scr2 = pscr.tile([128, M], BF, name="scr2", tag="scr", bufs=12)
                    scr3 = pscr.tile([128, M], BF, name="scr3", tag="scr", bufs=12)
                    # psa's ONLY consumer is this fast cast (slot frees in
                    # ~0.5us); nt2/sq_t come from the bf16 SBUF copy instead.
                    nc.vector.tensor_copy(at_sb[:, 0:W], psa[:, 0:W])
                    nc.scalar.activation(
                        scr1[:, 0:W], psb[:, 0:W], ACTF.Square,
                        accum_out=ns2b[:, idx : idx + 1],
                    )
                    # psb-freeing product first; the PSUM-independent at^2
                    # stat last so slots recycle as early as possible
                    nc.vector.scalar_tensor_tensor(
                        out=scr3[:, 0:W],
                        in0=psb[:, 0:W],
                        scalar=1.0,
                        in1=at_sb[:, 0:W],
                        op0=ALU.mult,
                        op1=ALU.mult,
                        accum_out=stb[:, idx : idx + 1],
                    )
                    nc.vector.scalar_tensor_tensor(
                        out=scr2[:, 0:W],
                        in0=at_sb[:, 0:W],
                        scalar=1.0,
                        in1=at_sb[:, 0:W],
                        op0=ALU.mult,
                        op1=ALU.mult,
                        accum_out=nt2b[:, idx : idx + 1],
                    )
                    if tile_hook is not None:
                        tile_hook(mt)
                    if W > 128:
                        g0 = 128 * (mt + 1)
                        segs = []  # aligned to PSUM bank boundaries
                        for bb0 in (0, 512):
                            lo = max(g0, bb0)
                            if lo < bb0 + 512:
                                segs.append((lo, bb0 + 512))

                        def _mk(mt=mt, segs=segs, scrs=(scr3, scr1, scr2)):
                            def emit():
                                for srow, scr in zip((0, 32, 64), scrs):
                                    for a2, b2 in segs:
                                        la = a2 - 128 * mt
                                        nc.tensor.matmul(
                                            cs[srow : srow + 2, a2:b2],
                                            lhsT=ones_r[:, 0:2],
                                            rhs=scr[:, la : la + b2 - a2],
                                            start=(mt == 0),
                                            stop=(mt == 6),
                                            skip_group_check=True,
                                        )
                            return emit

                        pending.append(_mk())
                for p in pending:
                    p()
                csb = pstat.tile([66, M], FP, name=f"csb_{bi}")
                nc.scalar.copy(csb[:], cs[:])
                for s in range(3):
                    nc.sync.dma_start(och[bi, s], csb[32 * s : 32 * s + 1, :])

            # ---- phase order: prep b0, SA b0 | IC | prep b1 (DMA), SA b1.
            # DMA program order on SP: F-b0, IC transposes, F-b1 -> PE is
            # never data-starved.
            prep_batch(0)

            # batch-1 prep is interleaved into SA-b0's per-tile stats: its F
            # loads dispatch right after b0's (slot-gated), and its DVE/ACT
            # work slots between SA-b0 stat ops without head-of-line blocking.
            _prep1 = [(tn, k) for tn in ("t", "s") for k in range(C // 128)]

            def _hook(mt):
                tn, k = _prep1[mt]
                prep_k(1, tn, k, tf if tn == "t" else sf)

            with tc.tile_pool(name="saps0", bufs=2, space="PSUM") as pps0:
                sa_batch(0, pps0, tile_hook=_hook)

            # IC loads: hardware DMA-transpose, [128, 4096] chunks land as
            # [128p, 32blk, 128ch] with m = blk*128 + p.
            lmats = {}
            for tname, hl in (("t", to), ("s", so)):
                tt = picl.tile([128, 128, 128], BF, name=f"tt_{tname}")
                for c in range(4):
                    nc.sync.dma_start_transpose(
                        tt[:, 32 * c : 32 * (c + 1), :],
                        hl[:, 4096 * c : 4096 * (c + 1)],
                    )
                lmats[tname] = tt

            # ---------------- IC phase (own PSUM pool) ----------------
            # One matmul per (tensor, m-block): the [128, 118] product holds
            # hi/lo cross-grams of both batches. The [21,21] block extraction
            # G = hh + hl + hl^T + ll and the loss math happen on the host
            # (f64) from the shipped OG blocks.
            with tc.tile_pool(name="icg", bufs=1, space="PSUM") as pg:
                for ti, tname in enumerate(("t", "s")):
                    tt = lmats[tname]
                    psg = pg.tile([48, 48], FP, name=f"psg_{tname}")
                    for j in range(128):
                        nc.tensor.matmul(
                            psg[:, :],
                            lhsT=tt[:, j, 0:48],
                            rhs=tt[:, j, 0:48],
                            start=(j == 0),
                            stop=(j == 127),
                        )
                    gsb = pstat.tile([48, 48], FP, name=f"gsb_{tname}")
                    nc.scalar.copy(gsb[:], psg[:])
                    nc.sync.dma_start(og[ti], gsb[:])

            # ---------------- SA batch 1 ----------------
            with tc.tile_pool(name="saps1", bufs=2, space="PSUM") as pps1:
                sa_batch(1, pps1)

            # ship raw SA stats; host does rho math in f64
            nc.sync.dma_start(ost[0], stb[:])
            nc.sync.dma_start(ost[1], ns2b[:])
            nc.sync.dma_start(ost[2], nt2b[:])

    _split_sync_waits(nc)
    return nc


_NC = None


def _get_nc():
    global _NC
    if _NC is None:
        _NC = _build()
    return _NC


def _hl_pack(X):
    """[B, CC, M2] f32 -> per-core [128, M2] bf16 with rows
    0:21 b0-hi, 32:53 b0-lo, 64:85 b1-hi, 96:117 b1-lo."""
    bf = ml_dtypes.bfloat16
    hi = X.astype(bf)
    out = np.zeros((N_CORES, 48, M2), dtype=bf)
    out[:, 0:21] = hi[0::2]
    out[:, 21:42] = hi[1::2]
    return out


def _make_in_maps(TF, SF, t_out, s_out):
    bf = ml_dtypes.bfloat16
    TFr = np.ascontiguousarray(np.asarray(TF, dtype=np.float32).reshape(B, C, M).astype(bf))
    SFr = np.ascontiguousarray(np.asarray(SF, dtype=np.float32).reshape(B, C, M).astype(bf))
    TOhl = _hl_pack(np.asarray(t_out, dtype=np.float32).reshape(B, CC, M2))
    SOhl = _hl_pack(np.asarray(s_out, dtype=np.float32).reshape(B, CC, M2))
    in_maps = []
    for i in range(N_CORES):
        sl = slice(BPC * i, BPC * (i + 1))
        in_maps.append(
            {
                "TF": np.ascontiguousarray(TFr[sl]),
                "SF": np.ascontiguousarray(SFr[sl]),
                "TOHL": np.ascontiguousarray(TOhl[i]),
                "SOHL": np.ascontiguousarray(SOhl[i]),
                "ONES": np.ones((128, 1), dtype=np.float32),
            }
        )
    return in_maps


def _assemble(results, s_out):
    EPS = 1e-12
    sa_rho = 0.0
    ic_num = 0.0
    for r in results:
        st = r["OST"][0].astype(np.float64)
        ns2 = r["OST"][1].astype(np.float64)
        nt2 = r["OST"][2].astype(np.float64)
        for bi in range(BPC):
            sl = slice(8 * bi, 8 * (bi + 1))
            ch = r["OCH"][bi].astype(np.float64)
            ch[:, 0:128] = 0.0  # chain cols 0:128 are uninitialized PSUM
            st[:, sl] += ch[0].reshape(8, 128).T
            ns2[:, sl] += ch[1].reshape(8, 128).T
            nt2[:, sl] += ch[2].reshape(8, 128).T
        sa_rho += float((st / np.maximum(np.sqrt(ns2 * nt2), EPS)).sum())
        gpair = []
        for ti in range(2):  # t, s
            gsb = r["OG"][ti].astype(np.float64)
            gb = [
                gsb[0:CC, 0:CC],
                gsb[CC : 2 * CC, CC : 2 * CC],
            ]
            gpair.append(gb)
        for bi in range(BPC):
            Gt, Gs = gpair[0][bi], gpair[1][bi]
            Gtn = Gt / np.maximum(
                np.linalg.norm(Gt, axis=1, keepdims=True), EPS
            )
            Gsn = Gs / np.maximum(
                np.linalg.norm(Gs, axis=1, keepdims=True), EPS
            )
            d = Gsn - Gtn
            ic_num += float((d * d).sum())
    sa_loss = (2.0 * B * M - 2.0 * sa_rho) / (B * M * M)
    ic_loss = ic_num / (CC * B)
    return (s_out, np.float32(ic_loss), np.float32(sa_loss))


def kernel(TF, SF, t_out, s_out, _trace=False):
    nc = _get_nc()
    in_maps = _make_in_maps(TF, SF, t_out, s_out)
    res = run_bass_kernel_spmd(nc, in_maps, core_ids=list(range(N_CORES)), trace=_trace)
    out = _assemble(res.results, s_out)
    if _trace:
        return out, res
    return out


# revision 52
# speedup vs baseline: 1.1510x; 1.0253x over previous
"""Trainium2 Bass kernel for nn_Distiller (attention-transfer distillation loss).

Computes on 8 NeuronCores (data-parallel over batch, 2 batches/core):
  SA part: per batch, weighted spatial grams A = V^T V with V = F * sqrt(Fc)
           (Fc = sum |F| over space), for teacher/student features; then
           rho_m = <A_S[m], A_T[m]> / (|A_S[m]| |A_T[m]|) per row.
           Uses the identity sum_n (Ahat_S - Ahat_T)^2 = 2 - 2*rho per row.
  IC part: per batch, channel grams G = L L^T of the [21, 16384] logit maps,
           same rho identity on the 21 rows of G.
Device emits per-row rho partial sums; host assembles the two scalar losses.
s_out passes through on host.
"""

import sys

if "/opt/trn_rl_repo" not in sys.path:
    sys.path.insert(0, "/opt/trn_rl_repo")

import numpy as np
import ml_dtypes

import concourse.bass as bass
import concourse.mybir as mybir
import concourse.tile as tile
from concourse.bass_utils import run_bass_kernel_spmd

# Problem shapes (hardcoded per spec)
B = 16
C = 512
M = 1024  # 32*32 spatial
CC = 21
M2 = 16384  # 128*128 spatial
N_CORES = 8
BPC = B // N_CORES  # batches per core = 2

FP = mybir.dt.float32
FPR = mybir.dt.float32r
BF = mybir.dt.bfloat16
AX = mybir.AxisListType.X
ALU = mybir.AluOpType
ACTF = mybir.ActivationFunctionType



def _split_sync_waits(nc, cap=1):
    """walrus in this container accepts at most `cap` sync waits per
    instruction; hoist excess waits onto same-engine NOPs just before."""
    n = 0
    for f in nc.m.functions:
        for bb in f.blocks:
            newlist = []
            for ins in bb.instructions:
                si = ins.sync_info
                if si is not None and si.on_wait and len(si.on_wait) > cap:
                    waits = list(si.on_wait)
                    hoist, keep = waits[:-cap], waits[-cap:]
                    for w in hoist:
                        n += 1
                        newlist.append(
                            mybir.InstNoOp(
                                name=f"waitsplit-{n}",
                                engine=ins.engine,
                                ins=[],
                                outs=[],
                                sync_info=mybir.SyncInfo(on_wait=[w], on_update=[]),
                            )
                        )
                    si.on_wait = keep
                newlist.append(ins)
            bb.instructions = newlist


def _build():
    nc = bass.Bass(trn_type="TRN2")
    # bf16 feature maps: halves the SA DMA bytes; V keeps f32r rounding
    tf = nc.dram_tensor("TF", [BPC, C, M], BF, kind="ExternalInput")
    sf = nc.dram_tensor("SF", [BPC, C, M], BF, kind="ExternalInput")
    # IC input: bf16 hi-only, teacher AND student packed into one tensor so
    # one matmul per m-block computes both grams. Rows: t_b0 0:21, t_b1
    # 21:42, s_b0 48:69, s_b1 69:90, zeros elsewhere.
    lhl = nc.dram_tensor("LHL", [96, M2], BF, kind="ExternalInput")
    ones_d = nc.dram_tensor("ONES", [128, 1], FP, kind="ExternalInput")
    ost = nc.dram_tensor("OST", [3, 128, 16], FP, kind="ExternalOutput")
    och = nc.dram_tensor("OCH", [BPC, 3, M], FP, kind="ExternalOutput")
    og = nc.dram_tensor("OG", [96, 96], FP, kind="ExternalOutput")

    with tile.TileContext(nc) as tc:
        with (
            tc.tile_pool(name="icl", bufs=1) as picl,
            tc.tile_pool(name="vt", bufs=1) as pv,
            tc.tile_pool(name="fstg", bufs=8) as pf,
            tc.tile_pool(name="scr", bufs=3) as pscr,
            tc.tile_pool(name="stat", bufs=1) as pstat,
        ):
            ones_f = pstat.tile([128, 2], FP, name="ones_f")
            nc.sync.dma_start(ones_f[:, 0:1], ones_d[:])
            nc.sync.dma_start(ones_f[:, 1:2], ones_d[:])
            ones_r = pstat.tile([128, 2], BF, name="ones_r")
            nc.vector.tensor_copy(ones_r[:], ones_f[:])

            vmats = {}
            fcs = {}

            def prep_k(bi, tname, k, src):
                # F staged in f32, V = F * sqrt(Fc) written as float32r (the
                # rounding producer the FP32r matmul verifier requires).
                key = (tname, bi)
                if key not in fcs:
                    fc = pstat.tile([128, 4], FP, name=f"fc_{tname}_{bi}")
                    sfc = pstat.tile([128, 4], FP, name=f"sfc_{tname}_{bi}")
                    fcs[key] = (fc, sfc)
                fc, sfc = fcs[key]
                f = pf.tile([128, M], BF, name=f"fstg_{k}", tag="fstg")
                nc.sync.dma_start(f[:], src[bi, 128 * k : 128 * (k + 1), :])
                nc.vector.tensor_reduce(
                    out=fc[:, k : k + 1],
                    in_=f[:],
                    axis=AX,
                    op=ALU.add,
                    apply_absolute_value=True,
                )
                nc.scalar.activation(
                    sfc[:, k : k + 1], fc[:, k : k + 1], ACTF.Sqrt
                )
                v = pv.tile([128, M], FPR, name=f"v_{tname}_{bi}_{k}")
                nc.vector.tensor_scalar_mul(v[:], f[:], sfc[:, k : k + 1])
                vmats[(tname, bi, k)] = v

            def prep_batch(bi):
                for tname, src in (("t", tf), ("s", sf)):
                    for k in range(C // 128):
                        prep_k(bi, tname, k, src)

            ns2b = pstat.tile([128, 16], FP, name="ns2b")
            nt2b = pstat.tile([128, 16], FP, name="nt2b")
            stb = pstat.tile([128, 16], FP, name="stb")

            def sa_batch(bi, pps, tile_hook=None):
                # A = V^T V is symmetric: compute only columns n >= the m-tile
                # diagonal. Row sums of the skipped lower triangle equal
                # column sums of the upper part, accumulated via ones^T @ f(A)
                # matmul chains into `cs` (partition 0=prod, 32=sqS, 64=sqT);
                # host merges. cs cols 0:128 are never written (garbage) and
                # are masked on the host.
                cs = pps.tile([66, M], FP, name="cs", tag="cs", bufs=1)
                pending = []
                for mt in range(8):
                    # emit chains deferred by THREE tiles so PE never waits
                    # on recent stats (scr pool holds 4 generations)
                    while len(pending) > 2:
                        pending.pop(0)()
                    idx = bi * 8 + mt
                    W = M - 128 * mt
                    psa = pps.tile([128, M], FP, name="psa", tag="psa", bufs=1)
                    psb = pps.tile([128, M], FP, name="psb", tag="psb", bufs=2)
                    # pieces must not cross the 512-elem PSUM bank boundary
                    pieces = [(0, 512), (512, W - 512)] if W > 512 else [(0, W)]
                    for dst, tname in ((psa, "t"), (psb, "s")):
                        for off, nn in pieces:
                            for k in range(4):
                                v = vmats[(tname, bi, k)]
                                nc.tensor.matmul(
                                    dst[:, off : off + nn],
                                    lhsT=v[:, 128 * mt : 128 * (mt + 1)],
                                    rhs=v[:, 128 * mt + off : 128 * mt + off + nn],
                                    start=(k == 0),
                                    stop=(k == 3),
                                )
                    at_sb = pscr.tile([128, M], BF, name="at_sb", tag="atsb")
                    scr1 = pscr.tile([128, M], BF, name="scr1", tag="scr", bufs=12)
                    scr2 = pscr.tile([128, M], BF, name="scr2", tag="scr", bufs=12)
                    scr3 = pscr.tile([128, M], BF, name="scr3", tag="scr", bufs=12)
                    # psa's ONLY consumer is this fast cast (slot frees in
                    # ~0.5us); nt2/sq_t come from the bf16 SBUF copy instead.
                    nc.vector.tensor_copy(at_sb[:, 0:W], psa[:, 0:W])
                    nc.scalar.activation(
                        scr1[:, 0:W], psb[:, 0:W], ACTF.Square,
                        accum_out=ns2b[:, idx : idx + 1],
                    )
                    # psb-freeing product first; the PSUM-independent at^2
                    # stat last so slots recycle as early as possible
                    nc.vector.scalar_tensor_tensor(
                        out=scr3[:, 0:W],
                        in0=psb[:, 0:W],
                        scalar=1.0,
                        in1=at_sb[:, 0:W],
                        op0=ALU.mult,
                        op1=ALU.mult,
                        accum_out=stb[:, idx : idx + 1],
                    )
                    nc.vector.scalar_tensor_tensor(
                        out=scr2[:, 0:W],
                        in0=at_sb[:, 0:W],
                        scalar=1.0,
                        in1=at_sb[:, 0:W],
                        op0=ALU.mult,
                        op1=ALU.mult,
                        accum_out=nt2b[:, idx : idx + 1],
                    )
                    if tile_hook is not None:
                        tile_hook(mt)
                    if W > 128:
                        g0 = 128 * (mt + 1)
                        segs = []  # aligned to PSUM bank boundaries
                        for bb0 in (0, 512):
                            lo = max(g0, bb0)
                            if lo < bb0 + 512:
                                segs.append((lo, bb0 + 512))

                        def _mk(mt=mt, segs=segs, scrs=(scr3, scr1, scr2)):
                            def emit():
                                for srow, scr in zip((0, 32, 64), scrs):
                                    for a2, b2 in segs:
                                        la = a2 - 128 * mt
                                        nc.tensor.matmul(
                                            cs[srow : srow + 2, a2:b2],
                                            lhsT=ones_r[:, 0:2],
                                            rhs=scr[:, la : la + b2 - a2],
                                            start=(mt == 0),
                                            stop=(mt == 6),
                                            skip_group_check=True,
                                        )
                            return emit

                        pending.append(_mk())
                for p in pending:
                    p()
                csb = pstat.tile([66, M], FP, name=f"csb_{bi}")
                nc.scalar.copy(csb[:], cs[:])
                for s in range(3):
                    nc.sync.dma_start(och[bi, s], csb[32 * s : 32 * s + 1, :])

            # ---- phase order: prep b0, SA b0 | IC | prep b1 (DMA), SA b1.
            # DMA program order on SP: F-b0, IC transposes, F-b1 -> PE is
            # never data-starved.
            prep_batch(0)

            # batch-1 prep is interleaved into SA-b0's per-tile stats: its F
            # loads dispatch right after b0's (slot-gated), and its DVE/ACT
            # work slots between SA-b0 stat ops without head-of-line blocking.
            _prep1 = [(tn, k) for tn in ("t", "s") for k in range(C // 128)]

            def _hook(mt):
                tn, k = _prep1[mt]
                prep_k(1, tn, k, tf if tn == "t" else sf)

            with tc.tile_pool(name="saps0", bufs=2, space="PSUM") as pps0:
                sa_batch(0, pps0, tile_hook=_hook)

            # IC loads: hardware DMA-transpose, [96, 4096] chunks land as
            # [128p, 32blk, 96ch] with m = blk*128 + p.
            tt_all = picl.tile([128, 128, 96], BF, name="tt_all")
            for c in range(4):
                nc.sync.dma_start_transpose(
                    tt_all[:, 32 * c : 32 * (c + 1), :],
                    lhl[:, 4096 * c : 4096 * (c + 1)],
                )

            # ---------------- IC phase (own PSUM pool) ----------------
            # One matmul per (tensor, m-block): the [128, 118] product holds
            # hi/lo cross-grams of both batches. The [21,21] block extraction
            # G = hh + hl + hl^T + ll and the loss math happen on the host
            # (f64) from the shipped OG blocks.
            with tc.tile_pool(name="icg", bufs=1, space="PSUM") as pg:
                psg = pg.tile([96, 96], FP, name="psg")
                for j in range(128):
                    nc.tensor.matmul(
                        psg[:, :],
                        lhsT=tt_all[:, j, 0:96],
                        rhs=tt_all[:, j, 0:96],
                        start=(j == 0),
                        stop=(j == 127),
                    )
                gsb = pstat.tile([96, 96], FP, name="gsb")
                nc.scalar.copy(gsb[:], psg[:])
                nc.sync.dma_start(og[:], gsb[:])

            # ---------------- SA batch 1 ----------------
            with tc.tile_pool(name="saps1", bufs=2, space="PSUM") as pps1:
                sa_batch(1, pps1)

            # ship raw SA stats; host does rho math in f64
            nc.sync.dma_start(ost[0], stb[:])
            nc.sync.dma_start(ost[1], ns2b[:])
            nc.sync.dma_start(ost[2], nt2b[:])

    _split_sync_waits(nc)
    return nc


_NC = None


def _get_nc():
    global _NC
    if _NC is None:
        _NC = _build()
    return _NC


def _hl_pack(T, S):
    """teacher+student [B, CC, M2] f32 -> per-core [96, M2] bf16; rows
    t_b0 0:21, t_b1 21:42, s_b0 48:69, s_b1 69:90."""
    bf = ml_dtypes.bfloat16
    th = T.astype(bf)
    sh = S.astype(bf)
    out = np.zeros((N_CORES, 96, M2), dtype=bf)
    out[:, 0:21] = th[0::2]
    out[:, 21:42] = th[1::2]
    out[:, 48:69] = sh[0::2]
    out[:, 69:90] = sh[1::2]
    return out


def _make_in_maps(TF, SF, t_out, s_out):
    bf = ml_dtypes.bfloat16
    TFr = np.ascontiguousarray(np.asarray(TF, dtype=np.float32).reshape(B, C, M).astype(bf))
    SFr = np.ascontiguousarray(np.asarray(SF, dtype=np.float32).reshape(B, C, M).astype(bf))
    LHL = _hl_pack(
        np.asarray(t_out, dtype=np.float32).reshape(B, CC, M2),
        np.asarray(s_out, dtype=np.float32).reshape(B, CC, M2),
    )
    in_maps = []
    for i in range(N_CORES):
        sl = slice(BPC * i, BPC * (i + 1))
        in_maps.append(
            {
                "TF": np.ascontiguousarray(TFr[sl]),
                "SF": np.ascontiguousarray(SFr[sl]),
                "LHL": np.ascontiguousarray(LHL[i]),
                "ONES": np.ones((128, 1), dtype=np.float32),
            }
        )
    return in_maps


def _assemble(results, s_out):
    EPS = 1e-12
    sa_rho = 0.0
    ic_num = 0.0
    for r in results:
        st = r["OST"][0].astype(np.float64)
        ns2 = r["OST"][1].astype(np.float64)
        nt2 = r["OST"][2].astype(np.float64)
        for bi in range(BPC):
            sl = slice(8 * bi, 8 * (bi + 1))
            ch = r["OCH"][bi].astype(np.float64)
            ch[:, 0:128] = 0.0  # chain cols 0:128 are uninitialized PSUM
            st[:, sl] += ch[0].reshape(8, 128).T
            ns2[:, sl] += ch[1].reshape(8, 128).T
            nt2[:, sl] += ch[2].reshape(8, 128).T
        sa_rho += float((st / np.maximum(np.sqrt(ns2 * nt2), EPS)).sum())
        gsb = r["OG"].astype(np.float64)
        gpair = [
            [gsb[0:CC, 0:CC], gsb[CC : 2 * CC, CC : 2 * CC]],
            [gsb[48 : 48 + CC, 48 : 48 + CC],
             gsb[48 + CC : 48 + 2 * CC, 48 + CC : 48 + 2 * CC]],
        ]
        for bi in range(BPC):
            Gt, Gs = gpair[0][bi], gpair[1][bi]
            Gtn = Gt / np.maximum(
                np.linalg.norm(Gt, axis=1, keepdims=True), EPS
            )
            Gsn = Gs / np.maximum(
                np.linalg.norm(Gs, axis=1, keepdims=True), EPS
            )
            d = Gsn - Gtn
            ic_num += float((d * d).sum())
    sa_loss = (2.0 * B * M - 2.0 * sa_rho) / (B * M * M)
    ic_loss = ic_num / (CC * B)
    return (s_out, np.float32(ic_loss), np.float32(sa_loss))


def kernel(TF, SF, t_out, s_out, _trace=False):
    nc = _get_nc()
    in_maps = _make_in_maps(TF, SF, t_out, s_out)
    res = run_bass_kernel_spmd(nc, in_maps, core_ids=list(range(N_CORES)), trace=_trace)
    out = _assemble(res.results, s_out)
    if _trace:
        return out, res
    return out
